# revision 20
# baseline (speedup 1.0000x reference)
"""GPT2 attention (B=2,S=2048,D=1024,H=16,hd=64, no causal mask) on 8 trn2 cores.

Sharding: core c handles batch b=c//4 and head-group g=c%4 (4 heads).
All device data is bf16 (halves transfer + SBUF footprint vs fp32);
matmul accumulation stays fp32 in PSUM. Host pre-transposes hidden
states (hidT upload) so no PE transposes are needed, pre-scales w_q by
1/sqrt(hd), and sums the 4 partial c_proj outputs per batch.

The attention middle section is paced by the ACT engine's exp
throughput (~1.1us per [128,1024] block). All remaining work (leftover
Q/K projections and the output projection) is queued as small "filler"
closures (two matmuls sharing one stationary operand, to amortize
LDWEIGHTS) and drained a few per k-tile, away from block boundaries,
so the PE stays continuously busy (HAM holds K=8/8) without starving
the ACT pipeline. hidT DMAs are split into column chunks and ordered
by first use so head compute starts ~2us in.

Per-core dataflow:
  hidT [1024,2048] bf16 (uploaded transposed)
  Q^T,K^T feature-major [128,2048] tiles (2 heads/tile) = w_chunk.T @ hidT
  V seq-major via matmul (lhsT=hidT chunk): vaug [k,65] blocks (col 64=1 for
    the softmax denominator, via one memset of the whole vaug to 1.0)
  per (head-pair p, 512-wide q chunk, 16 k tiles):
    S^T[k,q] for both heads with row-tiled concurrent matmuls (head A rows
    0-63, head B rows 64-127) into one [128,1024] fp32 PSUM tile
    one ACT exp per [128,1024] block -> bf16 SBUF
    O^T[65,512] per head += vaug.T @ E  (row 64 = softmax denominator)
    normalize: DVE copy to SBUF staging, fast reciprocal of denoms, GPSIMD
    partition_broadcast, DVE multiply -> obar pair-stacked [128,2048] bf16
    (head B's rows bounced to partitions 64-127 via SBUF->SBUF DMA)
  out[q,1024] = sum_p obar_pair_p.T @ wp_pair_p  (K=128, 2 matmuls)
"""

import sys

import numpy as np

if "/opt/trn_rl_repo" not in sys.path:
    sys.path.insert(0, "/opt/trn_rl_repo")

S = 2048
D = 1024
P = 128
NH = 4  # heads per core
HD = 64
N_CORES = 8

_CACHE = {}


def _build_program():
    import concourse.mybir as mybir
    from concourse import bacc
    from concourse.tile import TileContext

    bf16 = mybir.dt.bfloat16
    f32 = mybir.dt.float32
    AF = mybir.ActivationFunctionType
    ALU = mybir.AluOpType

    nc = bacc.Bacc(None, target_bir_lowering=False, debug=False)
    hidT = nc.declare_dram_parameter("hidT", [D, S], bf16, isOutput=False)
    wqkv = nc.declare_dram_parameter("wqkv", [D, 3 * NH * HD], bf16, isOutput=False)
    wp = nc.declare_dram_parameter("wp", [NH * HD, D], bf16, isOutput=False)
    out = nc.declare_dram_parameter("out", [S, D], bf16, isOutput=True)

    with TileContext(nc) as tc:
        with tc.tile_pool(name="persist", bufs=1) as per, \
             tc.tile_pool(name="ebp", bufs=2) as ebp, \
             tc.tile_pool(name="stgp", bufs=2) as stgp, \
             tc.tile_pool(name="nrm", bufs=2) as nrm, \
             tc.tile_pool(name="outp", bufs=4) as outp, \
             tc.tile_pool(name="psum", bufs=1, space="PSUM") as psum:
            # V (seq-major) + ones column per (head, ktile): 65-wide blocks
            vaug = per.tile([P, NH * 16 * 65], bf16)
            nc.gpsimd.memset(vaug[:], 1.0)
            # Q^T/K^T feature-major, 2 heads per tile: 0=Q01 1=Q23 2=K01 3=K23
            qkT = [per.tile([P, S], bf16, name=f"qkT{i}") for i in range(4)]
            # normalized attention output^T, pair-stacked (A rows 0:64, B 64:128)
            obar = [per.tile([P, S], bf16, name=f"obar{i}") for i in range(2)]
            wp_sb = [per.tile([P, D], bf16, name=f"wp{i}") for i in range(2)]
            hT = [per.tile([P, S], bf16, name=f"hT{i}") for i in range(8)]
            w_sb = [per.tile([P, 768], bf16, name=f"w{i}") for i in range(8)]
            # DMA order = first-use order: w + first two hidT column chunks
            # feed the K01 chains; the rest follows; wp only needed at the end
            for i in range(8):
                nc.sync.dma_start(out=w_sb[i][:], in_=wqkv[i * P : (i + 1) * P, :])
                nc.sync.dma_start(
                    out=hT[i][:, 0:1024], in_=hidT[i * P : (i + 1) * P, 0:1024]
                )
            for i in range(8):
                nc.sync.dma_start(
                    out=hT[i][:, 1024:2048], in_=hidT[i * P : (i + 1) * P, 1024:2048]
                )
            for p in range(2):
                nc.sync.dma_start(out=wp_sb[p][:], in_=wp[p * P : (p + 1) * P, :])

            # HAM pre-warm: ~3.5us of dummy matmuls while the input DMAs land,
            # so the real head work runs at 2.4GHz instead of the cold 1.2
            warm_src = per.tile([P, 512], bf16, name="warm_src")
            nc.gpsimd.memset(warm_src[:], 0.0)

            def warm_mms(n):
                wps_ = psum.tile([P, 512], f32, tag="aux", bufs=2, name="warmps")
                for _ in range(n):
                    nc.tensor.matmul(
                        wps_[:], lhsT=warm_src[:, 0:P], rhs=warm_src[:],
                        start=True, stop=True,
                    )

            warm_mms(18)

            def qk_single(ct, qc, emit=None):
                box = {}

                def mm(dt_):
                    def go():
                        if dt_ == 0:
                            box["ps"] = psum.tile(
                                [P, 512], f32, tag="aux", bufs=2, name="qkps"
                            )
                        nc.tensor.matmul(
                            box["ps"][:],
                            lhsT=w_sb[dt_][:, ct * P : (ct + 1) * P],
                            rhs=hT[dt_][:, qc * 512 : (qc + 1) * 512],
                            start=(dt_ == 0),
                            stop=(dt_ == 7),
                        )
                    return go

                def fin():
                    nc.vector.tensor_copy(
                        qkT[ct][:, qc * 512 : (qc + 1) * 512], box["ps"][:]
                    )

                closures = [mm(dt_) for dt_ in range(8)] + [fin]
                if emit is None:
                    for c in closures:
                        c()
                else:
                    emit.extend(closures)

            def v_unit(kt):
                # V rows kt*128.. for all 4 heads, scattered into vaug
                ps = psum.tile([P, 512], f32, tag="aux", bufs=2, name="vps")
                for dt_ in range(8):
                    nc.tensor.matmul(
                        ps[:, 0 : NH * HD],
                        lhsT=hT[dt_][:, kt * P : (kt + 1) * P],
                        rhs=w_sb[dt_][:, 512:768],
                        start=(dt_ == 0),
                        stop=(dt_ == 7),
                    )
                for h in range(NH):
                    base = (h * 16 + kt) * 65
                    nc.vector.tensor_copy(
                        vaug[:, base : base + HD], ps[:, h * HD : (h + 1) * HD]
                    )

            def c_unit(qt, emit):
                # out rows qt*128.. = sum_p obar_pair_p^T @ wp_pair_p
                box = {}

                def start_qt():
                    box["ot"] = outp.tile([P, D], bf16, tag="ot", name="ot")

                def mm(ec, p):
                    def go():
                        if p == 0:
                            box[ec] = psum.tile(
                                [P, 512], f32, tag="aux", bufs=2, name="cps"
                            )
                        nc.tensor.matmul(
                            box[ec][:],
                            lhsT=obar[p][:, qt * P : (qt + 1) * P],
                            rhs=wp_sb[p][:, ec * 512 : (ec + 1) * 512],
                            start=(p == 0),
                            stop=(p == 1),
                        )
                    return go

                def fin(ec):
                    def go():
                        nc.vector.tensor_copy(
                            box["ot"][:, ec * 512 : (ec + 1) * 512], box[ec][:]
                        )
                    return go

                def ship():
                    nc.sync.dma_start(
                        out=out[qt * P : (qt + 1) * P, :], in_=box["ot"][:]
                    )

                emit.extend([
                    start_qt,
                    mm(0, 0), mm(0, 1), fin(0),
                    mm(1, 0), mm(1, 1), fin(1),
                    ship,
                ])

            def b_block(p, qc, fillers, per_kt):
                hA, hB = 2 * p, 2 * p + 1
                qT, kT = qkT[p], qkT[2 + p]
                q0 = qc * 512
                qs = slice(q0, q0 + 512)
                opq = psum.tile([65, 1024], f32, tag="op", bufs=1, name="opq")
                for kt in range(16):
                    ks = slice(kt * P, (kt + 1) * P)
                    sp = psum.tile([P, 1024], f32, tag="sp", bufs=2, name="sp")
                    # row-tiled concurrent pair: A rows 0-63, B rows 64-127
                    nc.tensor.matmul(
                        sp[:, 0:512], lhsT=kT[0:HD, ks], rhs=qT[0:HD, qs],
                        start=True, stop=True,
                    )
                    nc.tensor.matmul(
                        sp[:, 512:1024], lhsT=kT[HD:P, ks], rhs=qT[HD:P, qs],
                        start=True, stop=True,
                    )
                    eb = ebp.tile([P, 1024], bf16, tag="eb", name="eb")
                    nc.scalar.activation(eb[:], sp[:], AF.Exp)
                    bA = (hA * 16 + kt) * 65
                    bB = (hB * 16 + kt) * 65
                    nc.tensor.matmul(
                        opq[:, 0:512], lhsT=vaug[:, bA : bA + 65],
                        rhs=eb[:, 0:512], start=(kt == 0), stop=(kt == 15),
                    )
                    nc.tensor.matmul(
                        opq[:, 512:1024], lhsT=vaug[:, bB : bB + 65],
                        rhs=eb[:, 512:1024], start=(kt == 0), stop=(kt == 15),
                    )
                    # keep the first/last k-tile filler-free so block
                    # boundaries stay tight
                    if 1 <= kt <= 14:
                        for _ in range(per_kt):
                            if fillers:
                                fillers.pop(0)()
                # normalize both heads: evacuate PSUM with ONE copy (frees the
                # op banks for the next block's AV), then SBUF-side math
                stg = stgp.tile([65, 1024], f32, tag="stg", name="stg")
                nc.vector.tensor_copy(stg[:], opq[:])
                # custom-DVE ops cannot shift partitions: bounce the denom row
                # (partition 64) to partition 0 with a plain copy first
                den = nrm.tile([1, 1024], f32, tag="den", name="den")
                nc.vector.tensor_copy(den[:], stg[64:65, :])
                rcp = nrm.tile([1, 1024], f32, tag="rcp", name="rcp")
                nc.vector.reciprocal_approx_fast(rcp[:], den[:])
                rbc = nrm.tile([HD, 1024], f32, tag="rbc", name="rbc")
                nc.gpsimd.partition_broadcast(rbc[:], rcp[0:1, :])
                with nc.allow_low_precision(reason="softmax normalize bf16"):
                    nc.vector.tensor_tensor(
                        out=obar[p][0:HD, qs], in0=stg[0:HD, 0:512],
                        in1=rbc[:, 0:512], op=ALU.mult,
                    )
                    btmp = nrm.tile([HD, 512], bf16, tag="btmp", name="btmp")
                    nc.vector.tensor_tensor(
                        out=btmp[:], in0=stg[0:HD, 512:1024],
                        in1=rbc[:, 512:1024], op=ALU.mult,
                    )
                # head B lives on partitions 64-127 of the pair-stacked obar;
                # DVE cannot shift partitions, SBUF->SBUF DMA can
                nc.sync.dma_start(out=obar[p][HD:P, qs], in_=btmp[:])

            # ---- stage A head: K01 (all chunks), Q01 chunk 0, V (all) ----
            for qc in range(4):
                qk_single(2, qc)
            qk_single(0, 0)
            for kt in range(16):
                v_unit(kt)

            # ---- pair 0, with remaining A-stage work as fine-grained filler
            # queue order respects first use: Q01 qc1/qc2 drain within
            # (p0,qc0); Q01 qc3 by (p0,qc1); K23/Q23 well before pair 1
            f_q = []
            for ct, qc in [(0, 1), (0, 2), (0, 3), (3, 0), (3, 1), (3, 2),
                           (3, 3), (1, 0), (1, 1), (1, 2), (1, 3)]:
                qk_single(ct, qc, emit=f_q)
            for qc in range(4):
                b_block(0, qc, f_q, per_kt=2)

            # ---- pair 1: projection as filler ----
            for qc in range(4):
                b_block(1, qc, f_q, per_kt=3)
                # projection for the q-range this qc completed (both pairs
                # done for qc by now) feeds the following blocks as filler
                for qt in range(4 * qc, 4 * qc + 4):
                    c_unit(qt, f_q)
            # hold HAM warm through the final normalize chain so the tail
            # projection runs at full clock
            warm_mms(14)
            # tail: whatever filler never got consumed (last C chunks)
            for f in f_q:
                f()

    nc.compile()
    return nc


def _get_nc():
    if "nc" not in _CACHE:
        _CACHE["nc"] = _build_program()
    return _CACHE["nc"]


def _shard_inputs(hidden_states, w_attn, w_proj):
    import ml_dtypes

    bf16 = ml_dtypes.bfloat16
    scale = 1.0 / np.sqrt(np.float32(HD))
    hidT_b = [
        np.ascontiguousarray(hidden_states[b].T).astype(bf16) for b in range(2)
    ]
    in_maps = []
    for c in range(N_CORES):
        b, g = divmod(c, 4)
        cs = slice(g * NH * HD, (g + 1) * NH * HD)
        wq = w_attn[:, 0:D][:, cs] * scale
        wk = w_attn[:, D : 2 * D][:, cs]
        wv = w_attn[:, 2 * D : 3 * D][:, cs]
        in_maps.append(
            {
                "hidT": hidT_b[b],
                "wqkv": np.ascontiguousarray(
                    np.concatenate([wq, wk, wv], axis=1)
                ).astype(bf16),
                "wp": np.ascontiguousarray(w_proj[cs, :]).astype(bf16),
            }
        )
    return in_maps


def run(hidden_states, w_attn, w_proj, trace=False):
    from concourse.bass_utils import run_bass_kernel_spmd

    nc = _get_nc()
    in_maps = _shard_inputs(hidden_states, w_attn, w_proj)
    res = run_bass_kernel_spmd(nc, in_maps, list(range(N_CORES)), trace=trace)
    parts = [res.results[c]["out"].astype(np.float32) for c in range(N_CORES)]
    out = np.stack(
        [
            parts[0] + parts[1] + parts[2] + parts[3],
            parts[4] + parts[5] + parts[6] + parts[7],
        ]
    ).astype(np.float32)
    return out, res


def kernel(hidden_states, w_attn, w_proj):
    out, _ = run(
        np.asarray(hidden_states), np.asarray(w_attn), np.asarray(w_proj)
    )
    return out
